# revision 8
# baseline (speedup 1.0000x reference)
"""Paged GQA decode attention (sparse_attention) on 8 TRN2 NeuronCores.

Sharding: tensor-parallel by KV head (8 heads -> 8 cores). Each core gets its
head's slice of the KV pool as combined bf16 rows [K(128)|V(128)] = 512B so a
single dma_gather descriptor per token fetches both K and V at the efficient
512B descriptor size (sub-512B descriptors pay a 2x DMA latency multiplier).

Per core dataflow (program fully specialized at build time on seq_lens meta,
identical across cores):
  gather: one transposed dma_gather per group-half -> plane0 = K^T [d,s]
          (directly usable by QK), plane1 = V^T [d,s].
  QK:     scores^T[s,4] = K^T_slot.T @ q_cols  (bf16, PSUM f32)
  exp:    one ACT Exp per group bank -> phi (bf16) in SBUF
  V^T->V: PE transpose per 128-token slot into bf16 PSUM banks (8 slots per
          bank), evacuated to SBUF by DVE/ACT alternately.
  PV:     o^T[4,128] += phi_slot @ V_slot  (bf16, PSUM f32 accum)
  sums:   ones^T @ phi -> per-slot-group softmax partial sums (one matmul
          per group); host does the final normalization.

Padding: slots are padded per request-half to 128 tokens using a zeroed,
unreferenced pool row. K=0 => score 0 => exp=1 exactly; V=0 contributes
nothing; the host subtracts the exact pad count from each request's softmax
denominator. No masking work on device.
"""

import numpy as np
import ml_dtypes

import concourse.bacc as bacc
import concourse.mybir as mybir
import concourse.tile as tile
from concourse.bass_utils import run_bass_kernel_spmd

B, S, HQ, HKV, D, G = 32, 2048, 32, 8, 128, 4
POOL = B * S
HALF = POOL // 2
SCALE = D ** -0.5
NCORES = 8
GROUPS = 8
RPG = B // GROUPS  # requests per group

BF16 = ml_dtypes.bfloat16

_prog_cache: dict = {}
LAST_RESULT = None  # test.py introspection (exec time etc.)


def _pad128(n):
    return (n + 127) // 128 * 128


def _layout(meta):
    """meta[g][h][j] = valid token count of request j in half h of group g.

    Returns per-group bookkeeping: per half (padded token count n, slot count,
    idx col offset, per-request slot starts/counts), global slot tables
    (owner request, half, local slot), and per-request global slot ranges.
    """
    info = []
    icol = 0  # running column offset into the merged idx tensor
    for g in range(GROUPS):
        halves = []
        for h in (0, 1):
            secs = meta[g][h]
            starts, slot_cnt = [], []
            pos = 0
            for j in range(RPG):
                starts.append(pos // 128)
                slot_cnt.append(_pad128(secs[j]) // 128)
                pos += _pad128(secs[j])
            halves.append(dict(n=pos, slots=pos // 128, ioff=icol,
                               starts=starts, slot_cnt=slot_cnt, secs=secs))
            icol += pos // 16
        n_lo = halves[0]["slots"]
        nslots = n_lo + halves[1]["slots"]
        # global slot -> (half, local slot, owner request)
        slot_map = []
        for h in (0, 1):
            hh = halves[h]
            for j in range(RPG):
                for li in range(hh["slot_cnt"][j]):
                    slot_map.append((h, hh["starts"][j] + li, j))
        # per request: list of (gslot_start, cnt) ranges
        req_ranges = []
        for j in range(RPG):
            rr = []
            for h in (0, 1):
                hh = halves[h]
                base = 0 if h == 0 else n_lo
                if hh["slot_cnt"][j]:
                    rr.append((base + hh["starts"][j], hh["slot_cnt"][j]))
            req_ranges.append(rr)
        info.append(dict(halves=halves, nslots=nslots,
                         slot_map=slot_map, req_ranges=req_ranges))
    return info, icol


def _build_program(meta):
    info, idx_w = _layout(meta)
    dt = mybir.dt
    nc = bacc.Bacc(trn_type="TRN2")

    kv_il = nc.dram_tensor("kv_il", [POOL, 256], dt.bfloat16, kind="ExternalInput")
    qT_d = nc.dram_tensor("qT", [128, 128], dt.bfloat16, kind="ExternalInput")
    ident_d = nc.dram_tensor("ident", [128, 128], dt.bfloat16, kind="ExternalInput")
    idx_w = max(1, idx_w)
    idx_d = nc.dram_tensor("idx_all", [128, idx_w], dt.int16, kind="ExternalInput")
    OC = RPG * D  # output cols per group
    o_dram = nc.dram_tensor("o_un", [G, B * D], dt.float32, kind="ExternalOutput")
    s_dram = nc.dram_tensor("sums", [GROUPS, 512], dt.float32, kind="ExternalOutput")

    with tile.TileContext(nc) as tc:
        with (
            tc.tile_pool(name="const", bufs=1) as cpool,
            tc.tile_pool(name="kvt", bufs=4) as kvtp,
            tc.tile_pool(name="vnat", bufs=2) as vnp,
            tc.tile_pool(name="phi", bufs=2) as php,
            tc.tile_pool(name="stg", bufs=2) as stgp,
            tc.tile_pool(name="ps_sc", bufs=2, space="PSUM") as pssc,
            tc.tile_pool(name="ps_sm", bufs=2, space="PSUM") as pssm,
            tc.tile_pool(name="ps_pv", bufs=2, space="PSUM") as pspv,
            tc.tile_pool(name="ps_vt", bufs=2, space="PSUM") as psvt,
        ):
            q_t = cpool.tile([128, 128], dt.bfloat16, tag="qT")
            id_t = cpool.tile([128, 128], dt.bfloat16, tag="ident")
            ones_t = cpool.tile([128, 1], dt.bfloat16, tag="ones")
            idx_t = cpool.tile([128, idx_w], dt.int16, tag="idxall")
            # per-group-half idx chunks: the first gather only waits for its
            # own slice instead of the whole index tensor
            for g in range(GROUPS):
                gi = info[g]
                for h in (0, 1):
                    n = gi["halves"][h]["n"]
                    if n == 0:
                        continue
                    i0 = gi["halves"][h]["ioff"]
                    nc.sync.dma_start(out=idx_t[:, i0:i0 + n // 16],
                                      in_=idx_d[:, i0:i0 + n // 16])
            nc.sync.dma_start(out=q_t[:], in_=qT_d[:])
            nc.sync.dma_start(out=id_t[:], in_=ident_d[:])
            nc.vector.memset(ones_t[:], 1.0)

            for g in range(GROUPS):
                gi = info[g]
                nslots = gi["nslots"]
                ncols = 4 * nslots
                if nslots == 0:
                    z = stgp.tile([G, OC], dt.float32, tag="ostg")
                    nc.vector.memset(z[:], 0.0)
                    nc.sync.dma_start(out=o_dram[0:G, OC * g:OC * (g + 1)],
                                      in_=z[:])
                    continue
                # --- gather combined K|V rows for both pool halves ---------
                kvt_tiles = {}
                for h in (0, 1):
                    n = gi["halves"][h]["n"]
                    if n == 0:
                        continue
                    ioff = gi["halves"][h]["ioff"]
                    it = idx_t[:, ioff:ioff + n // 16]
                    src = kv_il[0:HALF, :] if h == 0 else kv_il[HALF:POOL, :]
                    kvt = kvtp.tile([128, 2, n], dt.bfloat16, tag="kvt")
                    nc.gpsimd.dma_gather(
                        out_ap=kvt[:], in_ap=src, idxs_ap=it,
                        num_idxs=n, num_idxs_reg=n, elem_size=256,
                        transpose=True, single_packet=False)
                    kvt_tiles[h] = kvt

                # --- QK: scores^T into one PSUM bank -----------------------
                sc = pssc.tile([128, ncols], dt.float32, tag="sc")
                for s, (h, loc, j) in enumerate(gi["slot_map"]):
                    kvt = kvt_tiles[h]
                    kT = kvt[:, 0, 128 * loc:128 * (loc + 1)]
                    b = RPG * g + j
                    nc.tensor.matmul(sc[:, 4 * s:4 * s + 4], kT,
                                     q_t[:, 4 * b:4 * b + 4],
                                     start=True, stop=True)

                # --- softmax numerator (scores O(1); no max subtraction) ---
                phi = php.tile([128, ncols], dt.bfloat16, tag="phi")
                nc.scalar.activation(phi[:], sc[:],
                                     mybir.ActivationFunctionType.Exp)

                # --- V^T -> V via PE transpose, evacuate per PSUM bank -----
                vnat = vnp.tile([128, 128 * nslots], dt.bfloat16, tag="vnat")
                for c0 in range(0, nslots, 8):
                    cn = min(8, nslots - c0)
                    vtb = psvt.tile([128, 1024], dt.bfloat16, tag="vtb")
                    for s in range(c0, c0 + cn):
                        h, loc, j = gi["slot_map"][s]
                        vT = kvt_tiles[h][:, 1, 128 * loc:128 * (loc + 1)]
                        nc.tensor.matmul(vtb[:, 128 * (s - c0):128 * (s - c0 + 1)],
                                         vT, id_t[:], is_transpose=True,
                                         start=True, stop=True)
                    dst = vnat[:, 128 * c0:128 * (c0 + cn)]
                    if (c0 // 8) % 2 == 0:
                        nc.vector.tensor_copy(out=dst, in_=vtb[:, 0:128 * cn])
                    else:
                        nc.scalar.copy(out=dst, in_=vtb[:, 0:128 * cn])

                # --- PV: o^T accum per request -----------------------------
                pv = pspv.tile([G, OC], dt.float32, tag="pv")
                for j in range(RPG):
                    rr = gi["req_ranges"][j]
                    oc = 128 * j
                    if not rr:
                        nc.vector.memset(pv[0:G, oc:oc + 128], 0.0)
                        continue
                    tot = sum(cnt for _, cnt in rr)
                    si = 0
                    for (s0, cnt) in rr:
                        for li in range(cnt):
                            s = s0 + li
                            nc.tensor.matmul(
                                pv[0:G, oc:oc + 128],
                                phi[:, 4 * s:4 * s + 4],
                                vnat[:, 128 * s:128 * (s + 1)],
                                start=(si == 0), stop=(si == tot - 1))
                            si += 1

                # --- sums: one ones^T @ phi matmul per group ---------------
                sm = pssm.tile([1, ncols], dt.float32, tag="sm")
                nc.tensor.matmul(sm[0:1, 0:ncols], ones_t[:, 0:1],
                                 phi[:, 0:ncols], start=True, stop=True)

                ostg = stgp.tile([G, OC], dt.float32, tag="ostg")
                sstg = stgp.tile([1, 512], dt.float32, tag="sstg")
                nc.vector.tensor_copy(out=ostg[:], in_=pv[:])
                nc.vector.tensor_copy(out=sstg[0:1, 0:ncols],
                                      in_=sm[0:1, 0:ncols])
                nc.sync.dma_start(out=o_dram[0:G, OC * g:OC * (g + 1)],
                                  in_=ostg[:])
                nc.sync.dma_start(out=s_dram[g:g + 1, 0:ncols],
                                  in_=sstg[0:1, 0:ncols])

    nc.compile()
    return nc, info


def prepare(inputs):
    q = np.asarray(inputs["q"], np.float32)
    k = np.asarray(inputs["k"], np.float32)
    v = np.asarray(inputs["v"], np.float32)
    k_buffer = np.asarray(inputs["k_buffer"], np.float32)
    v_buffer = np.asarray(inputs["v_buffer"], np.float32)
    req_to_token = np.asarray(inputs["req_to_token"])
    req_pool_indices = np.asarray(inputs["req_pool_indices"])
    seq_lens = np.asarray(inputs["seq_lens"]).astype(np.int64)
    out_cache_loc = np.asarray(inputs["out_cache_loc"]).astype(np.int64)

    # store_kv_cache scatter (tiny: 32 rows) + per-request token lists
    kb = k_buffer.copy()
    vb = v_buffer.copy()
    kb[out_cache_loc] = k.reshape(B, HKV, D)
    vb[out_cache_loc] = v.reshape(B, HKV, D)
    tok = req_to_token[req_pool_indices]

    # one unreferenced pool row per half as the zero pad target
    used = np.zeros(POOL, bool)
    for b in range(B):
        used[tok[b, :seq_lens[b]]] = True
    free_lo = np.flatnonzero(~used[:HALF])
    free_hi = np.flatnonzero(~used[HALF:]) + HALF
    assert len(free_lo) and len(free_hi), "no free pad row in a pool half"
    z_lo, z_hi = int(free_lo[0]), int(free_hi[0])

    # second-smallest group first (fast pipeline fill), smallest last (short
    # drain tail), the rest biggest-first in between
    asc = list(np.argsort(seq_lens, kind="stable"))
    head, tail_, mid = asc[RPG:2 * RPG], asc[:RPG], asc[2 * RPG:][::-1]
    order = np.array(head + mid + tail_, dtype=np.int64)

    meta = []
    idx_blocks = []
    npad = np.zeros(B, np.int64)  # per processing-position pad token count
    for g in range(GROUPS):
        halves_secs = []
        for h in (0, 1):
            parts = []
            secs = []
            for j in range(RPG):
                pos = RPG * g + j
                b = int(order[pos])
                t = tok[b, :seq_lens[b]].astype(np.int64)
                tl = t[t < HALF] if h == 0 else t[t >= HALF] - HALF
                secs.append(len(tl))
                padded = _pad128(len(tl))
                npad[pos] += padded - len(tl)
                arr = np.full(padded, (z_lo if h == 0 else z_hi - HALF),
                              np.int64)
                arr[:len(tl)] = tl
                parts.append(arr)
            halves_secs.append(tuple(secs))
            full = np.concatenate(parts) if parts else np.zeros(0, np.int64)
            if len(full):
                # [16, n/16] wrap, replicated into all 8 GPSIMD-core stripes
                idx_blocks.append(
                    np.tile(full.astype(np.int16).reshape(-1, 16).T, (8, 1)))
        meta.append(tuple(halves_secs))
    meta = tuple(meta)
    if idx_blocks:
        idx_all = np.ascontiguousarray(np.concatenate(idx_blocks, axis=1))
    else:
        idx_all = np.zeros((128, 1), np.int16)

    if meta not in _prog_cache:
        _prog_cache[meta] = _build_program(meta)
    nc, info = _prog_cache[meta]

    ident = np.eye(128, dtype=BF16)
    in_maps = []
    for c in range(NCORES):
        kh = kb[:, c, :].astype(BF16)
        vh = vb[:, c, :].astype(BF16)
        kv = np.concatenate([kh, vh], axis=1)
        kv[z_lo] = 0
        kv[z_hi] = 0
        qc = (q.reshape(B, HKV, G, D)[order, c] * SCALE).reshape(B * G, D)
        qT = np.ascontiguousarray(qc.T).astype(BF16)
        im = {
            "kv_il": np.ascontiguousarray(kv),
            "qT": qT,
            "ident": ident,
            "idx_all": idx_all,
        }
        in_maps.append(im)
    return nc, info, in_maps, order, npad


def postprocess(results, info, order, npad, cores=None):
    OC = RPG * D
    out = np.zeros((B, HQ, D), np.float32)
    for c in (cores if cores is not None else range(NCORES)):
        o_un = results[c]["o_un"]
        sums = results[c]["sums"]
        for g in range(GROUPS):
            gi = info[g]
            for j in range(RPG):
                pos = RPG * g + j
                b = int(order[pos])
                stot = np.zeros(G, np.float64)
                for (s0, cnt) in gi["req_ranges"][j]:
                    seg = sums[g, 4 * s0:4 * (s0 + cnt)].astype(np.float64)
                    stot += seg.reshape(cnt, G).sum(axis=0)
                stot -= npad[pos]  # pad tokens contribute exp(0)=1 each
                ov = o_un[:, OC * g + 128 * j:OC * g + 128 * (j + 1)]
                with np.errstate(divide="ignore", invalid="ignore"):
                    out[b, c * G:(c + 1) * G, :] = ov / stot[:, None]
    return out.reshape(B, HQ * D).astype(np.float32)


def kernel(**inputs):
    global LAST_RESULT
    nc, info, in_maps, order, npad = prepare(inputs)
    res = run_bass_kernel_spmd(nc, in_maps, core_ids=list(range(NCORES)),
                               trace=False)
    LAST_RESULT = res
    return postprocess(res.results, info, order, npad)


# revision 9
# speedup vs baseline: 1.1103x; 1.1103x over previous
"""Paged GQA decode attention (sparse_attention) on 8 TRN2 NeuronCores.

Sharding: tensor-parallel by KV head (8 heads -> 8 cores). Each core gets its
head's slice of the KV pool as combined bf16 rows [K(128)|V(128)] = 512B so a
single dma_gather descriptor per token fetches both K and V at the efficient
512B descriptor size (sub-512B descriptors pay a 2x DMA latency multiplier).

Per core dataflow (program fully specialized at build time on seq_lens meta,
identical across cores):
  gather: one transposed dma_gather per group-half -> plane0 = K^T [d,s]
          (directly usable by QK), plane1 = V^T [d,s].
  QK:     scores^T[s,4] = K^T_slot.T @ q_cols  (bf16, PSUM f32)
  exp:    one ACT Exp per group bank -> phi (bf16) in SBUF
  V^T->V: PE transpose per 128-token slot into bf16 PSUM banks (8 slots per
          bank), evacuated to SBUF by DVE/ACT alternately.
  PV:     o^T[4,128] += phi_slot @ V_slot  (bf16, PSUM f32 accum)
  sums:   ones^T @ phi -> per-slot-group softmax partial sums (one matmul
          per group); host does the final normalization.

Padding: slots are padded per request-half to 128 tokens using a zeroed,
unreferenced pool row. K=0 => score 0 => exp=1 exactly; V=0 contributes
nothing; the host subtracts the exact pad count from each request's softmax
denominator. No masking work on device.
"""

import numpy as np
import ml_dtypes

import concourse.bacc as bacc
import concourse.mybir as mybir
import concourse.tile as tile
from concourse.bass_utils import run_bass_kernel_spmd

B, S, HQ, HKV, D, G = 32, 2048, 32, 8, 128, 4
POOL = B * S
HALF = POOL // 2
SCALE = D ** -0.5
NCORES = 8
GROUPS = 8
RPG = B // GROUPS  # requests per group

BF16 = ml_dtypes.bfloat16

_prog_cache: dict = {}
LAST_RESULT = None  # test.py introspection (exec time etc.)


def _pad128(n):
    return (n + 127) // 128 * 128


def _layout(meta):
    """meta[g][h][j] = valid token count of request j in half h of group g.

    Returns per-group bookkeeping: per half (padded token count n, slot count,
    idx col offset, per-request slot starts/counts), global slot tables
    (owner request, half, local slot), and per-request global slot ranges.
    """
    info = []
    icol = 0  # running column offset into the merged idx tensor
    for g in range(GROUPS):
        halves = []
        for h in (0, 1):
            secs = meta[g][h]
            starts, slot_cnt = [], []
            pos = 0
            for j in range(RPG):
                starts.append(pos // 128)
                slot_cnt.append(_pad128(secs[j]) // 128)
                pos += _pad128(secs[j])
            halves.append(dict(n=pos, slots=pos // 128, ioff=icol,
                               starts=starts, slot_cnt=slot_cnt, secs=secs))
            icol += pos // 16
        n_lo = halves[0]["slots"]
        nslots = n_lo + halves[1]["slots"]
        # global slot -> (half, local slot, owner request)
        slot_map = []
        for h in (0, 1):
            hh = halves[h]
            for j in range(RPG):
                for li in range(hh["slot_cnt"][j]):
                    slot_map.append((h, hh["starts"][j] + li, j))
        # per request: list of (gslot_start, cnt) ranges
        req_ranges = []
        for j in range(RPG):
            rr = []
            for h in (0, 1):
                hh = halves[h]
                base = 0 if h == 0 else n_lo
                if hh["slot_cnt"][j]:
                    rr.append((base + hh["starts"][j], hh["slot_cnt"][j]))
            req_ranges.append(rr)
        info.append(dict(halves=halves, nslots=nslots,
                         slot_map=slot_map, req_ranges=req_ranges))
    return info, icol


def _build_program(meta):
    info, idx_w = _layout(meta)
    dt = mybir.dt
    nc = bacc.Bacc(trn_type="TRN2")

    kv_il = nc.dram_tensor("kv_il", [POOL, 256], dt.bfloat16, kind="ExternalInput")
    qT_d = nc.dram_tensor("qT", [128, 128], dt.bfloat16, kind="ExternalInput")
    ident_d = nc.dram_tensor("ident", [128, 128], dt.bfloat16, kind="ExternalInput")
    idx_w = max(1, idx_w)
    idx_d = nc.dram_tensor("idx_all", [128, idx_w], dt.int16, kind="ExternalInput")
    OC = RPG * D  # output cols per group
    o_dram = nc.dram_tensor("o_un", [G, B * D], dt.float32, kind="ExternalOutput")
    s_dram = nc.dram_tensor("sums", [GROUPS, 512], dt.float32, kind="ExternalOutput")

    with tile.TileContext(nc) as tc:
        with (
            tc.tile_pool(name="const", bufs=1) as cpool,
            tc.tile_pool(name="kvt", bufs=4) as kvtp,
            tc.tile_pool(name="vnat", bufs=2) as vnp,
            tc.tile_pool(name="phi", bufs=2) as php,
            tc.tile_pool(name="stg", bufs=2) as stgp,
            tc.tile_pool(name="ps_sc", bufs=2, space="PSUM") as pssc,
            tc.tile_pool(name="ps_sm", bufs=2, space="PSUM") as pssm,
            tc.tile_pool(name="ps_pv", bufs=2, space="PSUM") as pspv,
            tc.tile_pool(name="ps_vt", bufs=2, space="PSUM") as psvt,
        ):
            q_t = cpool.tile([128, 128], dt.bfloat16, tag="qT")
            id_t = cpool.tile([128, 128], dt.bfloat16, tag="ident")
            ones_t = cpool.tile([128, 1], dt.bfloat16, tag="ones")
            idx_t = cpool.tile([128, idx_w], dt.int16, tag="idxall")
            # first group's idx slice first so its gather starts early
            _w0 = info[1]["halves"][0]["ioff"] if GROUPS > 1 else idx_w
            _w0 = max(1, min(_w0, idx_w))
            nc.sync.dma_start(out=idx_t[:, 0:_w0], in_=idx_d[:, 0:_w0])
            if _w0 < idx_w:
                nc.sync.dma_start(out=idx_t[:, _w0:idx_w], in_=idx_d[:, _w0:idx_w])
            nc.sync.dma_start(out=q_t[:], in_=qT_d[:])
            nc.sync.dma_start(out=id_t[:], in_=ident_d[:])
            nc.vector.memset(ones_t[:], 1.0)

            for g in range(GROUPS):
                gi = info[g]
                nslots = gi["nslots"]
                ncols = 4 * nslots
                if nslots == 0:
                    z = stgp.tile([G, OC], dt.float32, tag="ostg")
                    nc.vector.memset(z[:], 0.0)
                    nc.sync.dma_start(out=o_dram[0:G, OC * g:OC * (g + 1)],
                                      in_=z[:])
                    continue
                # --- gather combined K|V rows for both pool halves ---------
                kvt_tiles = {}
                for h in (0, 1):
                    n = gi["halves"][h]["n"]
                    if n == 0:
                        continue
                    ioff = gi["halves"][h]["ioff"]
                    it = idx_t[:, ioff:ioff + n // 16]
                    src = kv_il[0:HALF, :] if h == 0 else kv_il[HALF:POOL, :]
                    kvt = kvtp.tile([128, 2, n], dt.bfloat16, tag="kvt")
                    nc.gpsimd.dma_gather(
                        out_ap=kvt[:], in_ap=src, idxs_ap=it,
                        num_idxs=n, num_idxs_reg=n, elem_size=256,
                        transpose=True, single_packet=False)
                    kvt_tiles[h] = kvt

                # --- QK: scores^T into one PSUM bank -----------------------
                sc = pssc.tile([128, ncols], dt.float32, tag="sc")
                for s, (h, loc, j) in enumerate(gi["slot_map"]):
                    kvt = kvt_tiles[h]
                    kT = kvt[:, 0, 128 * loc:128 * (loc + 1)]
                    b = RPG * g + j
                    nc.tensor.matmul(sc[:, 4 * s:4 * s + 4], kT,
                                     q_t[:, 4 * b:4 * b + 4],
                                     start=True, stop=True)

                # --- softmax numerator (scores O(1); no max subtraction) ---
                phi = php.tile([128, ncols], dt.bfloat16, tag="phi")
                nc.scalar.activation(phi[:], sc[:],
                                     mybir.ActivationFunctionType.Exp)

                # --- V^T -> V via PE transpose, evacuate per PSUM bank -----
                vnat = vnp.tile([128, 128 * nslots], dt.bfloat16, tag="vnat")
                for c0 in range(0, nslots, 8):
                    cn = min(8, nslots - c0)
                    vtb = psvt.tile([128, 1024], dt.bfloat16, tag="vtb")
                    for s in range(c0, c0 + cn):
                        h, loc, j = gi["slot_map"][s]
                        vT = kvt_tiles[h][:, 1, 128 * loc:128 * (loc + 1)]
                        nc.tensor.matmul(vtb[:, 128 * (s - c0):128 * (s - c0 + 1)],
                                         vT, id_t[:], is_transpose=True,
                                         start=True, stop=True)
                    dst = vnat[:, 128 * c0:128 * (c0 + cn)]
                    if (c0 // 8) % 2 == 0:
                        nc.vector.tensor_copy(out=dst, in_=vtb[:, 0:128 * cn])
                    else:
                        nc.scalar.copy(out=dst, in_=vtb[:, 0:128 * cn])

                # --- PV: o^T accum per request -----------------------------
                pv = pspv.tile([G, OC], dt.float32, tag="pv")
                for j in range(RPG):
                    rr = gi["req_ranges"][j]
                    oc = 128 * j
                    if not rr:
                        nc.vector.memset(pv[0:G, oc:oc + 128], 0.0)
                        continue
                    tot = sum(cnt for _, cnt in rr)
                    si = 0
                    for (s0, cnt) in rr:
                        for li in range(cnt):
                            s = s0 + li
                            nc.tensor.matmul(
                                pv[0:G, oc:oc + 128],
                                phi[:, 4 * s:4 * s + 4],
                                vnat[:, 128 * s:128 * (s + 1)],
                                start=(si == 0), stop=(si == tot - 1))
                            si += 1

                # --- sums: one ones^T @ phi matmul per group ---------------
                sm = pssm.tile([1, ncols], dt.float32, tag="sm")
                nc.tensor.matmul(sm[0:1, 0:ncols], ones_t[:, 0:1],
                                 phi[:, 0:ncols], start=True, stop=True)

                ostg = stgp.tile([G, OC], dt.float32, tag="ostg")
                sstg = stgp.tile([1, 512], dt.float32, tag="sstg")
                nc.vector.tensor_copy(out=ostg[:], in_=pv[:])
                nc.vector.tensor_copy(out=sstg[0:1, 0:ncols],
                                      in_=sm[0:1, 0:ncols])
                nc.sync.dma_start(out=o_dram[0:G, OC * g:OC * (g + 1)],
                                  in_=ostg[:])
                nc.sync.dma_start(out=s_dram[g:g + 1, 0:ncols],
                                  in_=sstg[0:1, 0:ncols])

    nc.compile()
    return nc, info


def prepare(inputs):
    q = np.asarray(inputs["q"], np.float32)
    k = np.asarray(inputs["k"], np.float32)
    v = np.asarray(inputs["v"], np.float32)
    k_buffer = np.asarray(inputs["k_buffer"], np.float32)
    v_buffer = np.asarray(inputs["v_buffer"], np.float32)
    req_to_token = np.asarray(inputs["req_to_token"])
    req_pool_indices = np.asarray(inputs["req_pool_indices"])
    seq_lens = np.asarray(inputs["seq_lens"]).astype(np.int64)
    out_cache_loc = np.asarray(inputs["out_cache_loc"]).astype(np.int64)

    # store_kv_cache scatter (tiny: 32 rows) + per-request token lists
    kb = k_buffer.copy()
    vb = v_buffer.copy()
    kb[out_cache_loc] = k.reshape(B, HKV, D)
    vb[out_cache_loc] = v.reshape(B, HKV, D)
    tok = req_to_token[req_pool_indices]

    # one unreferenced pool row per half as the zero pad target
    used = np.zeros(POOL, bool)
    for b in range(B):
        used[tok[b, :seq_lens[b]]] = True
    free_lo = np.flatnonzero(~used[:HALF])
    free_hi = np.flatnonzero(~used[HALF:]) + HALF
    assert len(free_lo) and len(free_hi), "no free pad row in a pool half"
    z_lo, z_hi = int(free_lo[0]), int(free_hi[0])

    # second-smallest group first (fast pipeline fill), smallest last (short
    # drain tail), the rest biggest-first in between
    asc = list(np.argsort(seq_lens, kind="stable"))
    head, tail_, mid = asc[RPG:2 * RPG], asc[:RPG], asc[2 * RPG:][::-1]
    order = np.array(head + mid + tail_, dtype=np.int64)

    meta = []
    idx_blocks = []
    npad = np.zeros(B, np.int64)  # per processing-position pad token count
    for g in range(GROUPS):
        halves_secs = []
        for h in (0, 1):
            parts = []
            secs = []
            for j in range(RPG):
                pos = RPG * g + j
                b = int(order[pos])
                t = tok[b, :seq_lens[b]].astype(np.int64)
                tl = t[t < HALF] if h == 0 else t[t >= HALF] - HALF
                secs.append(len(tl))
                padded = _pad128(len(tl))
                npad[pos] += padded - len(tl)
                arr = np.full(padded, (z_lo if h == 0 else z_hi - HALF),
                              np.int64)
                arr[:len(tl)] = tl
                parts.append(arr)
            halves_secs.append(tuple(secs))
            full = np.concatenate(parts) if parts else np.zeros(0, np.int64)
            if len(full):
                # [16, n/16] wrap, replicated into all 8 GPSIMD-core stripes
                idx_blocks.append(
                    np.tile(full.astype(np.int16).reshape(-1, 16).T, (8, 1)))
        meta.append(tuple(halves_secs))
    meta = tuple(meta)
    if idx_blocks:
        idx_all = np.ascontiguousarray(np.concatenate(idx_blocks, axis=1))
    else:
        idx_all = np.zeros((128, 1), np.int16)

    if meta not in _prog_cache:
        _prog_cache[meta] = _build_program(meta)
    nc, info = _prog_cache[meta]

    ident = np.eye(128, dtype=BF16)
    in_maps = []
    for c in range(NCORES):
        kh = kb[:, c, :].astype(BF16)
        vh = vb[:, c, :].astype(BF16)
        kv = np.concatenate([kh, vh], axis=1)
        kv[z_lo] = 0
        kv[z_hi] = 0
        qc = (q.reshape(B, HKV, G, D)[order, c] * SCALE).reshape(B * G, D)
        qT = np.ascontiguousarray(qc.T).astype(BF16)
        im = {
            "kv_il": np.ascontiguousarray(kv),
            "qT": qT,
            "ident": ident,
            "idx_all": idx_all,
        }
        in_maps.append(im)
    return nc, info, in_maps, order, npad


def postprocess(results, info, order, npad, cores=None):
    OC = RPG * D
    out = np.zeros((B, HQ, D), np.float32)
    for c in (cores if cores is not None else range(NCORES)):
        o_un = results[c]["o_un"]
        sums = results[c]["sums"]
        for g in range(GROUPS):
            gi = info[g]
            for j in range(RPG):
                pos = RPG * g + j
                b = int(order[pos])
                stot = np.zeros(G, np.float64)
                for (s0, cnt) in gi["req_ranges"][j]:
                    seg = sums[g, 4 * s0:4 * (s0 + cnt)].astype(np.float64)
                    stot += seg.reshape(cnt, G).sum(axis=0)
                stot -= npad[pos]  # pad tokens contribute exp(0)=1 each
                ov = o_un[:, OC * g + 128 * j:OC * g + 128 * (j + 1)]
                with np.errstate(divide="ignore", invalid="ignore"):
                    out[b, c * G:(c + 1) * G, :] = ov / stot[:, None]
    return out.reshape(B, HQ * D).astype(np.float32)


def kernel(**inputs):
    global LAST_RESULT
    nc, info, in_maps, order, npad = prepare(inputs)
    res = run_bass_kernel_spmd(nc, in_maps, core_ids=list(range(NCORES)),
                               trace=False)
    LAST_RESULT = res
    return postprocess(res.results, info, order, npad)
